# revision 24
# baseline (speedup 1.0000x reference)
"""Trainium2 Bass kernel for nn_ComplexAttention (B=4,H=8,T=2048,D=256).

Strategy
--------
* Shard the 32 (b,h) pairs across 8 NeuronCores, 4 per core (data parallel).
* Algebraic fusion removes two of the five projection GEMM groups:
    - scores only need Re(q conj(k))^T, so the k-projection folds into the
      q side:  u = xq @ A + i(...) with A = Wq^T Wk + ..., and
      scores^T = xkr^T-chunks against u directly (per-q constants from b_k
      are softmax-invariant and dropped; b_q becomes a u-bias row).
    - the o-projection folds into v: veff = xkv @ (W_o W_v)^T, so the
      attention context matmul directly produces the (unnormalized) output.
* All matmuls in fp16 (same PE rate as bf16, ~4x lower quantization noise).
* Softmax denominator off the PE: DVE strided tensor_reduce accumulates the
  16 exp chunks, one small all-ones matmul broadcasts the cross-partition
  sum, DVE fast-reciprocal normalizes at the PSUM drain.
* Single ACT function table (natural_log_exp_and_others) for the whole
  kernel: sqrt(m2) = exp(0.5*ln(m2+eps)), sigmoid(z) = 1/(1+exp(-z)) with
  the reciprocal on DVE. No ACT_TABLE_LOAD churn.
* Software-pipelined emission: produce(t+1) interleaves with consume(t);
  the sums finalization for tile t+1 is injected between consume(t)'s
  context matmuls and its gate matmuls so the PE never waits on the DVE
  reduce chain.
"""

import numpy as np

B, H, T, D = 4, 8, 2048, 256
NCORES = 8
BH = B * H
BH_PER_CORE = BH // NCORES  # 4
P = 128
DC = D // P       # 2 chunks of the feature dim
QT = 512          # q-tile width (matmul free dim / PSUM bank)
NQT = T // QT     # 4 q-tiles
NKC = T // P      # 16 k-chunks
NTT = T // P      # 16 t-tiles for veff
EPS = 1e-8
SCALE = 1.0 / np.sqrt(D)

WNAMES = ["A", "Bm", "Bmn", "WerT", "WeiT", "WeinT", "gwT"]
NW = len(WNAMES)
BNAMES = ["bur", "bui", "bor", "boi", "bgn"]
NB = len(BNAMES)

F16 = np.float16

_BUILT = None  # cache so repeated kernel() calls reuse the compiled module


def _emit_kernel(nc, tc, ctx, tens):
    from concourse import mybir

    f32 = mybir.dt.float32
    fp16 = mybir.dt.float16
    AF = mybir.ActivationFunctionType

    consts = ctx.enter_context(tc.tile_pool(name="consts", bufs=1))
    inpool = ctx.enter_context(tc.tile_pool(name="inpool", bufs=2))
    upool = ctx.enter_context(tc.tile_pool(name="upool", bufs=1))
    vpool = ctx.enter_context(tc.tile_pool(name="vpool", bufs=2))
    attnpool = ctx.enter_context(tc.tile_pool(name="attnpool", bufs=2))
    sumpool = ctx.enter_context(tc.tile_pool(name="sumpool", bufs=2))
    outppool = ctx.enter_context(tc.tile_pool(name="outppool", bufs=2))
    m2pool = ctx.enter_context(tc.tile_pool(name="m2pool", bufs=1))
    smallpool = ctx.enter_context(tc.tile_pool(name="smallpool", bufs=1))
    ps_mm = ctx.enter_context(tc.tile_pool(name="ps_mm", bufs=4, space="PSUM"))
    ps_out = ctx.enter_context(tc.tile_pool(name="ps_out", bufs=2, space="PSUM"))
    ps_sums = ctx.enter_context(tc.tile_pool(name="ps_sums", bufs=2, space="PSUM"))

    # ---- constants: A first (first matmul's stationary), rest interleaved
    # with the first input load ----
    bpack = consts.tile([P, NB * DC], f32, tag="bpack")
    bias = {name: bpack[:, i * DC:(i + 1) * DC] for i, name in enumerate(BNAMES)}

    wpack = consts.tile([P, DC, NW * D], fp16, tag="wpack")
    wp_dram = tens["wpack"]
    nc.sync.dma_start(wpack[:, :, :D], wp_dram[:, :, :D])  # A only
    w = {name: wpack[:, :, i * D:(i + 1) * D] for i, name in enumerate(WNAMES)}

    def load_consts_mid():
        nc.sync.dma_start(wpack[:, :, D:2 * D], wp_dram[:, :, D:2 * D])  # Bm

    def load_consts_rest():
        nc.sync.dma_start(bpack[:], tens["bpack"][:])
        nc.sync.dma_start(wpack[:, :, 2 * D:], wp_dram[:, :, 2 * D:])

    ones_k = consts.tile([P, P], fp16, tag="ones_k")
    nc.vector.memset(ones_k[:], 1.0)
    eps_t = consts.tile([P, 1], f32, tag="eps")
    nc.vector.memset(eps_t[:], EPS)

    def load_inputs(bh, first=False):
        """DMA the 4 input tensors for one (b,h); xq in halves so the first
        u-projection matmuls can start as soon as possible."""
        with nc.named_scope(f"load{bh}"):
            # xq is dead after proj_u, long before the next load is emitted:
            # single-buffered (xkv stays double-buffered for the prefetch)
            xqr = inpool.tile([P, DC, T], fp16, tag="xqr", name="xqr", bufs=1)
            xqi = inpool.tile([P, DC, T], fp16, tag="xqi", name="xqi", bufs=1)
            xkr = inpool.tile([P, DC, T], fp16, tag="xkr", name="xkr")
            xki = inpool.tile([P, DC, T], fp16, tag="xki", name="xki")
            if first:
                # head-ordered: first u-projection tile's operands first, the
                # remaining constants slotted between input slices
                for t, name in ((xqr, "xq_r"), (xqi, "xq_i")):
                    nc.sync.dma_start(t[:, :, :QT], tens[name][bh][:, :, :QT])
                load_consts_mid()
                for t, name in ((xqr, "xq_r"), (xqi, "xq_i")):
                    nc.sync.dma_start(t[:, :, QT:2 * QT],
                                      tens[name][bh][:, :, QT:2 * QT])
                load_consts_rest()
                for t, name in ((xqr, "xq_r"), (xqi, "xq_i")):
                    nc.sync.dma_start(t[:, :, 2 * QT:],
                                      tens[name][bh][:, :, 2 * QT:])
            else:
                half = T // 2
                for t, name in ((xqr, "xq_r"), (xqi, "xq_i")):
                    nc.sync.dma_start(t[:, :, :half],
                                      tens[name][bh][:, :, :half])
                    nc.sync.dma_start(t[:, :, half:],
                                      tens[name][bh][:, :, half:])
            for t, name in ((xkr, "xkv_r"), (xki, "xkv_i")):
                nc.sync.dma_start(t[:], tens[name][bh][:])
        return {"xqr": xqr, "xqi": xqi, "xkr": xkr, "xki": xki}

    def proj_u(bh, ld):
        """u_r = xq_r@A + xq_i@Bm + bur ; u_i = xq_i@A - xq_r@Bm + bui.
        d-major output [P, DC, T] fp16 (weights stationary)."""
        xqr, xqi = ld["xqr"], ld["xqi"]
        with nc.named_scope(f"u{bh}"):
            u_r = upool.tile([P, DC, T], fp16, tag="u_r")
            u_i = upool.tile([P, DC, T], fp16, tag="u_i")
            for dst, s1, w1, s2, w2, b in (
                (u_r, xqr, "A", xqi, "Bm", "bur"),
                (u_i, xqi, "A", xqr, "Bmn", "bui"),
            ):
                for co in range(DC):
                    for tt in range(NQT):
                        ts = slice(tt * QT, (tt + 1) * QT)
                        ps = ps_mm.tile([P, QT], f32, tag="mm")
                        nc.tensor.matmul(ps, w[w1][:, 0, co * P:(co + 1) * P],
                                         s1[:, 0, ts], start=True, stop=False)
                        nc.tensor.matmul(ps, w[w1][:, 1, co * P:(co + 1) * P],
                                         s1[:, 1, ts], start=False, stop=False)
                        nc.tensor.matmul(ps, w[w2][:, 0, co * P:(co + 1) * P],
                                         s2[:, 0, ts], start=False, stop=False)
                        nc.tensor.matmul(ps, w[w2][:, 1, co * P:(co + 1) * P],
                                         s2[:, 1, ts], start=False, stop=True)
                        nc.scalar.activation(dst[:, co, ts], ps, AF.Identity,
                                             bias=bias[b][:, co:co + 1])
        return u_r, u_i

    def proj_veff(bh, ld):
        """veff = xkv @ (W_o W_v)^T in t-major layout [P, NTT, D] fp16
        (x stationary)."""
        xkr, xki = ld["xkr"], ld["xki"]
        with nc.named_scope(f"veff{bh}"):
            v_r = vpool.tile([P, NTT, D], fp16, tag="v_r")
            v_i = vpool.tile([P, NTT, D], fp16, tag="v_i")
            for tt in range(NTT):
                tsl = slice(tt * P, (tt + 1) * P)
                for dst, s1, w1, s2, w2 in (
                    (v_r, xkr, "WerT", xki, "WeinT"),
                    (v_i, xki, "WerT", xkr, "WeiT"),
                ):
                    ps = ps_mm.tile([P, QT], f32, tag="mm")
                    pv = ps[:, :D]
                    nc.tensor.matmul(pv, s1[:, 0, tsl], w[w1][:, 0, :],
                                     start=True, stop=False)
                    nc.tensor.matmul(pv, s1[:, 1, tsl], w[w1][:, 1, :],
                                     start=False, stop=False)
                    nc.tensor.matmul(pv, s2[:, 0, tsl], w[w2][:, 0, :],
                                     start=False, stop=False)
                    nc.tensor.matmul(pv, s2[:, 1, tsl], w[w2][:, 1, :],
                                     start=False, stop=True)
                    nc.scalar.copy(dst[:, tt, :], pv)
        return v_r, v_i

    def produce(st, qt):
        """scoresT chunks -> exp -> DVE partial k-sums (fp16)."""
        u_r, u_i = st["u_r"], st["u_i"]
        xkr, xki = st["xkr"], st["xki"]
        qsl = slice(qt * QT, (qt + 1) * QT)
        with nc.named_scope(f"attn{st['bh']}_{qt}"):
            attn = attnpool.tile([P, NKC, QT], fp16, tag="attn")
            tree = sumpool.tile([P, 8, QT], fp16, tag="tree")
            for kc in range(NKC):
                ksl = slice(kc * P, (kc + 1) * P)
                sc = ps_mm.tile([P, QT], f32, tag="mm")
                nc.tensor.matmul(sc, xkr[:, 0, ksl], u_r[:, 0, qsl],
                                 start=True, stop=False)
                nc.tensor.matmul(sc, xkr[:, 1, ksl], u_r[:, 1, qsl],
                                 start=False, stop=False)
                nc.tensor.matmul(sc, xki[:, 0, ksl], u_i[:, 0, qsl],
                                 start=False, stop=False)
                nc.tensor.matmul(sc, xki[:, 1, ksl], u_i[:, 1, qsl],
                                 start=False, stop=True)
                nc.scalar.activation(attn[:, kc, :], sc, AF.Exp, scale=SCALE)
                # partial softmax sums: contiguous fp16 log-tree adds on DVE,
                # first half overlaps the second half's score matmuls
                if kc == 7:
                    nc.vector.tensor_add(tree[:, 0:4, :], attn[:, 0:4, :],
                                         attn[:, 4:8, :])
                elif kc == NKC - 1:
                    nc.vector.tensor_add(tree[:, 4:8, :], attn[:, 8:12, :],
                                         attn[:, 12:16, :])
            for width in (4, 2, 1):
                nc.vector.tensor_add(tree[:, 0:width, :], tree[:, 0:width, :],
                                     tree[:, width:2 * width, :])
        return {"attn": attn, "p16": tree[:, 0, :], "qt": qt, "bh": st["bh"],
                "st": st}

    def sums_mm(pr):
        """all-ones matmul broadcasts the cross-partition sum into PSUM."""
        with nc.named_scope(f"sums{pr['bh']}_{pr['qt']}"):
            sums = ps_sums.tile([P, QT], f32, tag="sums")
            nc.tensor.matmul(sums, ones_k[:], pr["p16"],
                             start=True, stop=True)
        pr["sums"] = sums

    def sums_recip(pr):
        """DVE fast reciprocal -> bc (per-q normalization, partition-bcast)."""
        # bc lives from consume(t-1) (where it is computed) into consume(t):
        # needs its own double buffer
        bc = smallpool.tile([P, QT], f32, tag="bc", name="bc", bufs=2)
        nc.vector.reciprocal_approx_fast(bc[:], pr["sums"][:])
        pr["bc"] = bc

    def sums_fin(pr):
        sums_mm(pr)
        sums_recip(pr)

    def consume_out(pr, then_pr=None, qoff=0, qw=QT):
        """B-phase: context/output matmuls -> normalize -> bias-add + square
        drains -> m2, for columns [qoff, qoff+qw). then_pr: the next tile's
        sums finalization, injected after the context matmuls (PE) / after
        m2add (DVE). Tiles allocate on the first call and are shared by
        later column-slices (tail split)."""
        st, qt, bh = pr["st"], pr["qt"], pr["bh"]
        attn, bc = pr["attn"], pr["bc"]
        v_r, v_i = st["v_r"], st["v_i"]
        csl = slice(qoff, qoff + qw)
        with nc.named_scope(f"out{bh}_{qt}"):
            if "out_r" not in pr:
                pr["out_r"] = outppool.tile([P, DC, QT], f32, tag="out_r",
                                            name="out_r")
                pr["out_i"] = outppool.tile([P, DC, QT], f32, tag="out_i",
                                            name="out_i")
                pr["m2"] = m2pool.tile([P, DC, QT], f32, tag="m2r",
                                       name="m2r", bufs=2)
                pr["m2i"] = m2pool.tile([P, DC, QT], f32, tag="m2i",
                                        name="m2i")
                pr["tmp"] = smallpool.tile([P, DC, QT], f32, tag="tmp",
                                           name="tmp")
            out_r = pr["out_r"][:, :, csl]
            out_i = pr["out_i"][:, :, csl]
            m2r = pr["m2"][:, :, csl]
            m2i = pr["m2i"][:, :, csl]
            tmp = pr["tmp"][:, :, csl]
            for dst, m2, vsrc, b in ((out_r, m2r, v_r, "bor"),
                                     (out_i, m2i, v_i, "boi")):
                for c in range(DC):
                    cps = ps_out.tile([P, QT], f32, tag="outp",
                                      name="outp")[:, :qw]
                    for kc in range(NKC):
                        nc.tensor.matmul(cps, vsrc[:, kc, c * P:(c + 1) * P],
                                         attn[:, kc, csl],
                                         start=(kc == 0), stop=(kc == NKC - 1),
                                         skip_group_check=True)
                    # normalize, then bias-add on ACT (Identity) + square (for
                    # the magnitude) in the same table
                    nc.vector.tensor_mul(tmp[:, c, :], cps, bc[:, csl])
                    nc.scalar.activation(dst[:, c, :], tmp[:, c, :],
                                         AF.Identity, bias=bias[b][:, c:c + 1])
                    nc.scalar.activation(m2[:, c, :], tmp[:, c, :],
                                         AF.Square, bias=bias[b][:, c:c + 1])

            if then_pr is not None:
                sums_mm(then_pr)  # PE: right after the context matmuls

            nc.vector.tensor_add(m2r[:], m2r[:], m2i[:])
            if then_pr is not None:
                sums_recip(then_pr)  # DVE: after m2add

    def consume_gate(pr, qoff=0, qw=QT):
        """C-phase (deferred one pipeline stage so the ACT table switches for
        Sqrt sit under the next tile's PE work): mag -> gate matmul ->
        sigmoid -> gated store, for columns [qoff, qoff+qw)."""
        qt, bh = pr["qt"], pr["bh"]
        out_r = pr["out_r"][:, :, qoff:qoff + qw]
        out_i = pr["out_i"][:, :, qoff:qoff + qw]
        m2 = pr["m2"][:, :, qoff:qoff + qw]
        with nc.named_scope(f"gate{bh}_{qt}"):
            mag = smallpool.tile([P, DC, QT], fp16, tag="mag", name="mag")[:, :, :qw]
            nc.scalar.activation(mag[:], m2[:], AF.Sqrt, bias=eps_t[:])

            gate = smallpool.tile([P, DC, QT], f32, tag="gate", name="gate")[:, :, :qw]
            eg = smallpool.tile([P, DC, QT], f32, tag="eg", name="eg")[:, :, :qw]
            for go in range(DC):
                gps = ps_mm.tile([P, QT], f32, tag="mm", name="mm")[:, :qw]
                nc.tensor.matmul(gps, w["gwT"][:, 0, go * P:(go + 1) * P],
                                 mag[:, 0, :], start=True, stop=False)
                nc.tensor.matmul(gps, w["gwT"][:, 1, go * P:(go + 1) * P],
                                 mag[:, 1, :], start=False, stop=True)
                # sigmoid(z+bg) = 1/(1 + exp(-z-bg)): exp on ACT (exp table),
                # +1 and reciprocal on DVE
                nc.scalar.activation(eg[:, go, :], gps, AF.Exp, scale=-1.0,
                                     bias=bias["bgn"][:, go:go + 1])
            nc.vector.tensor_scalar_add(eg[:], eg[:], 1.0)
            nc.vector.reciprocal_approx_fast(gate[:], eg[:])

            nc.vector.tensor_mul(out_r[:], out_r[:], gate[:])
            nc.vector.tensor_mul(out_i[:], out_i[:], gate[:])
            osl = slice(qt * QT + qoff, qt * QT + qoff + qw)
            for c in range(DC):
                nc.sync.dma_start(tens["yr"][bh, c, :, osl], out_r[:, c, :])
                nc.sync.dma_start(tens["yi"][bh, c, :, osl], out_i[:, c, :])

    # ---- software-pipelined emission ---------------------------------------
    # steady state per cycle: produce(t) | B(t-1) [+ sums(t)] | C(t-2)
    pending = None       # produced, awaiting B
    pending_gate = None  # B done, awaiting C
    lds = {0: load_inputs(0, first=True)}
    for bh in range(BH_PER_CORE):
        ld = lds[bh]
        u_r, u_i = proj_u(bh, ld)
        v_r, v_i = proj_veff(bh, ld)
        st = {"bh": bh, "u_r": u_r, "u_i": u_i, "v_r": v_r, "v_i": v_i,
              "xkr": ld["xkr"], "xki": ld["xki"]}
        for qt in range(NQT):
            if bh == 0 and qt == 1:
                continue  # produced during warm-up below
            pr = produce(st, qt)
            if pending is None:
                # warm-up: emit the next produce before the first sums
                # finalization so the reduce-chain latency hides behind PE work
                pr2 = produce(st, 1)
                sums_fin(pr)
                consume_out(pr, then_pr=pr2)
                pending, pending_gate = pr2, pr
                continue
            if qt == 1 and bh + 1 < BH_PER_CORE:
                lds[bh + 1] = load_inputs(bh + 1)  # prefetch next (b,h)
            consume_out(pending, then_pr=pr)
            consume_gate(pending_gate)
            pending_gate, pending = pending, pr
        if bh == 0 and BH_PER_CORE > 1:
            lds[1] = load_inputs(1)  # prefetch (emitted after (0,3) produce)
    # drain the pipeline: split the final out-phase into column halves and
    # interleave the gate phases so PE work covers the ACT table switches
    consume_out(pending, qoff=0, qw=QT // 2)
    consume_gate(pending_gate)
    consume_out(pending, qoff=QT // 2, qw=QT // 2)
    consume_gate(pending, qoff=0, qw=QT // 2)
    consume_gate(pending, qoff=QT // 2, qw=QT // 2)


def _build():
    global _BUILT
    if _BUILT is not None:
        return _BUILT
    from contextlib import ExitStack
    import concourse.tile as tile
    from concourse import bacc, mybir

    f32 = mybir.dt.float32
    fp16 = mybir.dt.float16

    nc = bacc.Bacc("TRN2", target_bir_lowering=False, debug=False,
                   num_devices=NCORES)

    tens = {}
    for name in ("xq_r", "xq_i", "xkv_r", "xkv_i"):
        tens[name] = nc.dram_tensor(name, [BH_PER_CORE, P, DC, T], fp16,
                                    kind="ExternalInput").ap()
    tens["wpack"] = nc.dram_tensor("wpack", [P, DC, NW * D], fp16,
                                   kind="ExternalInput").ap()
    tens["bpack"] = nc.dram_tensor("bpack", [P, NB * DC], f32,
                                   kind="ExternalInput").ap()
    for name in ("yr", "yi"):
        tens[name] = nc.dram_tensor(name, [BH_PER_CORE, DC, P, T], f32,
                                    kind="ExternalOutput").ap()

    with tile.TileContext(nc) as tc:
        with ExitStack() as ctx:
            _emit_kernel(nc, tc, ctx, tens)

    nc.compile()
    _BUILT = nc
    return nc


def _lhsT_pack(m):
    """[din, dout] stationary weight -> [P, DC, D] fp16 (din chunked,
    partition-major so the DMA is contiguous)."""
    return np.ascontiguousarray(
        m.reshape(DC, P, D).transpose(1, 0, 2).astype(F16))


def _bias_pack(b):
    """[D] bias -> [P, DC] f32 (per-partition d-major layout)."""
    return np.ascontiguousarray(np.asarray(b).reshape(DC, P).T.astype(np.float32))


def _x_pack(x):
    """[BH, T, D] fp32 -> [BH, P, DC, T] fp16 (d-major, partition-major so
    the DMA is contiguous per partition)."""
    xb = x.astype(F16)
    return np.ascontiguousarray(xb.reshape(BH, T, DC, P).transpose(0, 3, 2, 1))


def kernel(**inputs):
    inputs = {k: np.asarray(v) for k, v in inputs.items()}

    nc = _build()
    from concourse.bass_utils import run_bass_kernel_spmd

    xq_r = _x_pack(inputs["q_in_r"].reshape(BH, T, D))
    xq_i = _x_pack(inputs["q_in_i"].reshape(BH, T, D))
    xkv_r = _x_pack(inputs["kv_in_r"].reshape(BH, T, D))
    xkv_i = _x_pack(inputs["kv_in_i"].reshape(BH, T, D))

    f64 = np.float64
    qwr = inputs["q_wr"].astype(f64); qwi = inputs["q_wi"].astype(f64)
    kwr = inputs["k_wr"].astype(f64); kwi = inputs["k_wi"].astype(f64)
    vwr = inputs["v_wr"].astype(f64); vwi = inputs["v_wi"].astype(f64)
    owr = inputs["o_wr"].astype(f64); owi = inputs["o_wi"].astype(f64)
    gw = inputs["gate_w"].astype(f64)

    # u-projection: scores^T = (xq@A + xq_i@Bm ...) against raw kv inputs
    A = qwr.T @ kwr + qwi.T @ kwi
    Bm = qwr.T @ kwi - qwi.T @ kwr
    # veff: W_eff = W_o W_v (complex product)
    Wer = owr @ vwr - owi @ vwi
    Wei = owr @ vwi + owi @ vwr

    wmats = {"A": A, "Bm": Bm, "Bmn": -Bm,
             "WerT": Wer.T, "WeiT": Wei.T, "WeinT": -Wei.T, "gwT": gw.T}
    wpack = np.concatenate([_lhsT_pack(wmats[n]) for n in WNAMES], axis=-1)

    # u bias rows (b_q folded through the k weights; b_k drops: it only adds
    # per-q constants to scores, which softmax ignores)
    bqr = inputs["q_br"].astype(f64); bqi = inputs["q_bi"].astype(f64)
    bur = bqr @ kwr + bqi @ kwi
    bui = bqi @ kwr - bqr @ kwi
    # out bias: b_out = W_o b_v + b_o (v bias survives softmax row-sums = 1)
    vbr = inputs["v_br"].astype(f64); vbi = inputs["v_bi"].astype(f64)
    bor = inputs["o_br"].astype(f64) + owr @ vbr - owi @ vbi
    boi = inputs["o_bi"].astype(f64) + owi @ vbr + owr @ vbi

    bmats = {"bur": bur, "bui": bui, "bor": bor, "boi": boi,
             "bgn": -inputs["gate_b"].astype(f64)}
    bpack = np.concatenate([_bias_pack(bmats[n]) for n in BNAMES], axis=-1)

    consts = {"wpack": np.ascontiguousarray(wpack),
              "bpack": np.ascontiguousarray(bpack)}

    in_maps = []
    for c in range(NCORES):
        sl = slice(c * BH_PER_CORE, (c + 1) * BH_PER_CORE)
        m = dict(consts)
        m["xq_r"] = xq_r[sl]
        m["xq_i"] = xq_i[sl]
        m["xkv_r"] = xkv_r[sl]
        m["xkv_i"] = xkv_i[sl]
        in_maps.append(m)

    res = run_bass_kernel_spmd(nc, in_maps, core_ids=list(range(NCORES)))

    def unpack(name):
        full = np.concatenate([res.results[c][name] for c in range(NCORES)], axis=0)
        # [BH, DC, P, T] -> [BH, T, DC*P] -> [B, H, T, D]
        return np.ascontiguousarray(
            full.transpose(0, 3, 1, 2).reshape(B, H, T, D).astype(np.float32))

    return unpack("yr"), unpack("yi")


if __name__ == "__main__":
    # smoke test with random inputs
    rng = np.random.default_rng(0)
    fake = {}
    for nm in ("q_in_r", "q_in_i", "kv_in_r", "kv_in_i"):
        fake[nm] = rng.standard_normal((B, H, T, D), dtype=np.float32)
    for p in ("q", "k", "v", "o"):
        fake[f"{p}_wr"] = rng.standard_normal((D, D), dtype=np.float32) * 0.044
        fake[f"{p}_wi"] = rng.standard_normal((D, D), dtype=np.float32) * 0.044
        fake[f"{p}_br"] = np.zeros(D, np.float32)
        fake[f"{p}_bi"] = np.zeros(D, np.float32)
    fake["gate_w"] = rng.standard_normal((D, D), dtype=np.float32) * 0.044
    fake["gate_b"] = np.zeros(D, np.float32)
    yr, yi = kernel(**fake)
    print("OK", yr.shape, yi.shape, yr.dtype)


# revision 27
# speedup vs baseline: 1.0071x; 1.0071x over previous
"""Trainium2 Bass kernel for nn_ComplexAttention (B=4,H=8,T=2048,D=256).

Strategy
--------
* Shard the 32 (b,h) pairs across 8 NeuronCores, 4 per core (data parallel).
* Algebraic fusion removes two of the five projection GEMM groups:
    - scores only need Re(q conj(k))^T, so the k-projection folds into the
      q side:  u = xq @ A + i(...) with A = Wq^T Wk + ..., and
      scores^T = xkr^T-chunks against u directly (per-q constants from b_k
      are softmax-invariant and dropped; b_q becomes a u-bias row).
    - the o-projection folds into v: veff = xkv @ (W_o W_v)^T, so the
      attention context matmul directly produces the (unnormalized) output.
* All matmuls in fp16 (same PE rate as bf16, ~4x lower quantization noise).
* Softmax denominator off the PE: DVE strided tensor_reduce accumulates the
  16 exp chunks, one small all-ones matmul broadcasts the cross-partition
  sum, DVE fast-reciprocal normalizes at the PSUM drain.
* Single ACT function table (natural_log_exp_and_others) for the whole
  kernel: sqrt(m2) = exp(0.5*ln(m2+eps)), sigmoid(z) = 1/(1+exp(-z)) with
  the reciprocal on DVE. No ACT_TABLE_LOAD churn.
* Software-pipelined emission: produce(t+1) interleaves with consume(t);
  the sums finalization for tile t+1 is injected between consume(t)'s
  context matmuls and its gate matmuls so the PE never waits on the DVE
  reduce chain.
"""

import numpy as np

B, H, T, D = 4, 8, 2048, 256
NCORES = 8
BH = B * H
BH_PER_CORE = BH // NCORES  # 4
P = 128
DC = D // P       # 2 chunks of the feature dim
QT = 512          # q-tile width (matmul free dim / PSUM bank)
NQT = T // QT     # 4 q-tiles
NKC = T // P      # 16 k-chunks
NTT = T // P      # 16 t-tiles for veff
EPS = 1e-8
SCALE = 1.0 / np.sqrt(D)

WNAMES = ["A", "Bm", "Bmn", "WerT", "WeiT", "WeinT", "gwT"]
NW = len(WNAMES)
BNAMES = ["bur", "bui", "bor", "boi", "bgn"]
NB = len(BNAMES)

F16 = np.float16

_BUILT = None  # cache so repeated kernel() calls reuse the compiled module


def _emit_kernel(nc, tc, ctx, tens):
    from concourse import mybir

    f32 = mybir.dt.float32
    fp16 = mybir.dt.float16
    AF = mybir.ActivationFunctionType

    consts = ctx.enter_context(tc.tile_pool(name="consts", bufs=1))
    inpool = ctx.enter_context(tc.tile_pool(name="inpool", bufs=2))
    upool = ctx.enter_context(tc.tile_pool(name="upool", bufs=1))
    vpool = ctx.enter_context(tc.tile_pool(name="vpool", bufs=2))
    attnpool = ctx.enter_context(tc.tile_pool(name="attnpool", bufs=2))
    sumpool = ctx.enter_context(tc.tile_pool(name="sumpool", bufs=2))
    outppool = ctx.enter_context(tc.tile_pool(name="outppool", bufs=2))
    m2pool = ctx.enter_context(tc.tile_pool(name="m2pool", bufs=1))
    smallpool = ctx.enter_context(tc.tile_pool(name="smallpool", bufs=1))
    ps_mm = ctx.enter_context(tc.tile_pool(name="ps_mm", bufs=4, space="PSUM"))
    ps_out = ctx.enter_context(tc.tile_pool(name="ps_out", bufs=3, space="PSUM"))
    # sums(t) is drained (recip) before sums(t+1)'s ones-matmul is even
    # emitted: one bank suffices
    ps_sums = ctx.enter_context(tc.tile_pool(name="ps_sums", bufs=1, space="PSUM"))

    # ---- constants: A first (first matmul's stationary), rest interleaved
    # with the first input load ----
    bpack = consts.tile([P, NB * DC], f32, tag="bpack")
    bias = {name: bpack[:, i * DC:(i + 1) * DC] for i, name in enumerate(BNAMES)}

    wpack = consts.tile([P, DC, NW * D], fp16, tag="wpack")
    wp_dram = tens["wpack"]
    nc.sync.dma_start(wpack[:, :, :D], wp_dram[:, :, :D])  # A only
    w = {name: wpack[:, :, i * D:(i + 1) * D] for i, name in enumerate(WNAMES)}

    def load_consts_mid():
        nc.sync.dma_start(wpack[:, :, D:2 * D], wp_dram[:, :, D:2 * D])  # Bm

    def load_consts_rest():
        nc.sync.dma_start(bpack[:], tens["bpack"][:])
        nc.sync.dma_start(wpack[:, :, 2 * D:], wp_dram[:, :, 2 * D:])

    ones_k = consts.tile([P, P], fp16, tag="ones_k")
    nc.vector.memset(ones_k[:], 1.0)
    eps_t = consts.tile([P, 1], f32, tag="eps")
    nc.vector.memset(eps_t[:], EPS)

    def load_inputs(bh, first=False):
        """DMA the 4 input tensors for one (b,h); xq in halves so the first
        u-projection matmuls can start as soon as possible."""
        with nc.named_scope(f"load{bh}"):
            # xq is dead after proj_u, long before the next load is emitted:
            # single-buffered (xkv stays double-buffered for the prefetch)
            xqr = inpool.tile([P, DC, T], fp16, tag="xqr", name="xqr", bufs=1)
            xqi = inpool.tile([P, DC, T], fp16, tag="xqi", name="xqi", bufs=1)
            xkr = inpool.tile([P, DC, T], fp16, tag="xkr", name="xkr")
            xki = inpool.tile([P, DC, T], fp16, tag="xki", name="xki")
            if first:
                # head-ordered: first u-projection tile's operands first, the
                # remaining constants slotted between input slices; spread
                # across idle engine queues so DGE issue happens in parallel
                nc.sync.dma_start(xqr[:, :, :QT], tens["xq_r"][bh][:, :, :QT])
                nc.scalar.dma_start(xqi[:, :, :QT], tens["xq_i"][bh][:, :, :QT])
                load_consts_mid()
                nc.gpsimd.dma_start(xqr[:, :, QT:2 * QT],
                                    tens["xq_r"][bh][:, :, QT:2 * QT])
                nc.scalar.dma_start(xqi[:, :, QT:2 * QT],
                                    tens["xq_i"][bh][:, :, QT:2 * QT])
                load_consts_rest()
                nc.sync.dma_start(xqr[:, :, 2 * QT:],
                                  tens["xq_r"][bh][:, :, 2 * QT:])
                nc.gpsimd.dma_start(xqi[:, :, 2 * QT:],
                                    tens["xq_i"][bh][:, :, 2 * QT:])
            else:
                half = T // 2
                for t, name in ((xqr, "xq_r"), (xqi, "xq_i")):
                    nc.sync.dma_start(t[:, :, :half],
                                      tens[name][bh][:, :, :half])
                    nc.sync.dma_start(t[:, :, half:],
                                      tens[name][bh][:, :, half:])
            for t, name in ((xkr, "xkv_r"), (xki, "xkv_i")):
                nc.sync.dma_start(t[:], tens[name][bh][:])
        return {"xqr": xqr, "xqi": xqi, "xkr": xkr, "xki": xki}

    def proj_u(bh, ld):
        """u_r = xq_r@A + xq_i@Bm + bur ; u_i = xq_i@A - xq_r@Bm + bui.
        d-major output [P, DC, T] fp16 (weights stationary)."""
        xqr, xqi = ld["xqr"], ld["xqi"]
        with nc.named_scope(f"u{bh}"):
            u_r = upool.tile([P, DC, T], fp16, tag="u_r")
            u_i = upool.tile([P, DC, T], fp16, tag="u_i")
            for dst, s1, w1, s2, w2, b in (
                (u_r, xqr, "A", xqi, "Bm", "bur"),
                (u_i, xqi, "A", xqr, "Bmn", "bui"),
            ):
                for co in range(DC):
                    for tt in range(NQT):
                        ts = slice(tt * QT, (tt + 1) * QT)
                        ps = ps_mm.tile([P, QT], f32, tag="mm")
                        nc.tensor.matmul(ps, w[w1][:, 0, co * P:(co + 1) * P],
                                         s1[:, 0, ts], start=True, stop=False)
                        nc.tensor.matmul(ps, w[w1][:, 1, co * P:(co + 1) * P],
                                         s1[:, 1, ts], start=False, stop=False)
                        nc.tensor.matmul(ps, w[w2][:, 0, co * P:(co + 1) * P],
                                         s2[:, 0, ts], start=False, stop=False)
                        nc.tensor.matmul(ps, w[w2][:, 1, co * P:(co + 1) * P],
                                         s2[:, 1, ts], start=False, stop=True)
                        nc.scalar.activation(dst[:, co, ts], ps, AF.Identity,
                                             bias=bias[b][:, co:co + 1])
        return u_r, u_i

    def proj_veff(bh, ld):
        """veff = xkv @ (W_o W_v)^T in t-major layout [P, NTT, D] fp16
        (x stationary)."""
        xkr, xki = ld["xkr"], ld["xki"]
        with nc.named_scope(f"veff{bh}"):
            v_r = vpool.tile([P, NTT, D], fp16, tag="v_r")
            v_i = vpool.tile([P, NTT, D], fp16, tag="v_i")
            for tt in range(NTT):
                tsl = slice(tt * P, (tt + 1) * P)
                for dst, s1, w1, s2, w2 in (
                    (v_r, xkr, "WerT", xki, "WeinT"),
                    (v_i, xki, "WerT", xkr, "WeiT"),
                ):
                    ps = ps_mm.tile([P, QT], f32, tag="mm")
                    pv = ps[:, :D]
                    nc.tensor.matmul(pv, s1[:, 0, tsl], w[w1][:, 0, :],
                                     start=True, stop=False)
                    nc.tensor.matmul(pv, s1[:, 1, tsl], w[w1][:, 1, :],
                                     start=False, stop=False)
                    nc.tensor.matmul(pv, s2[:, 0, tsl], w[w2][:, 0, :],
                                     start=False, stop=False)
                    nc.tensor.matmul(pv, s2[:, 1, tsl], w[w2][:, 1, :],
                                     start=False, stop=True)
                    nc.scalar.copy(dst[:, tt, :], pv)
        return v_r, v_i

    def produce(st, qt):
        """scoresT chunks -> exp -> DVE partial k-sums (fp16)."""
        u_r, u_i = st["u_r"], st["u_i"]
        xkr, xki = st["xkr"], st["xki"]
        qsl = slice(qt * QT, (qt + 1) * QT)
        with nc.named_scope(f"attn{st['bh']}_{qt}"):
            attn = attnpool.tile([P, NKC, QT], fp16, tag="attn")
            tree = sumpool.tile([P, 8, QT], fp16, tag="tree")
            for kc in range(NKC):
                ksl = slice(kc * P, (kc + 1) * P)
                sc = ps_mm.tile([P, QT], f32, tag="mm")
                nc.tensor.matmul(sc, xkr[:, 0, ksl], u_r[:, 0, qsl],
                                 start=True, stop=False)
                nc.tensor.matmul(sc, xkr[:, 1, ksl], u_r[:, 1, qsl],
                                 start=False, stop=False)
                nc.tensor.matmul(sc, xki[:, 0, ksl], u_i[:, 0, qsl],
                                 start=False, stop=False)
                nc.tensor.matmul(sc, xki[:, 1, ksl], u_i[:, 1, qsl],
                                 start=False, stop=True)
                nc.scalar.activation(attn[:, kc, :], sc, AF.Exp, scale=SCALE)
                # partial softmax sums: contiguous fp16 log-tree adds on DVE,
                # first half overlaps the second half's score matmuls
                if kc == 7:
                    nc.vector.tensor_add(tree[:, 0:4, :], attn[:, 0:4, :],
                                         attn[:, 4:8, :])
                elif kc == NKC - 1:
                    nc.vector.tensor_add(tree[:, 4:8, :], attn[:, 8:12, :],
                                         attn[:, 12:16, :])
            for width in (4, 2, 1):
                nc.vector.tensor_add(tree[:, 0:width, :], tree[:, 0:width, :],
                                     tree[:, width:2 * width, :])
        return {"attn": attn, "p16": tree[:, 0, :], "qt": qt, "bh": st["bh"],
                "st": st}

    def sums_mm(pr):
        """all-ones matmul broadcasts the cross-partition sum into PSUM."""
        with nc.named_scope(f"sums{pr['bh']}_{pr['qt']}"):
            sums = ps_sums.tile([P, QT], f32, tag="sums")
            nc.tensor.matmul(sums, ones_k[:], pr["p16"],
                             start=True, stop=True)
        pr["sums"] = sums

    def sums_recip(pr):
        """DVE fast reciprocal -> bc (per-q normalization, partition-bcast)."""
        # bc lives from consume(t-1) (where it is computed) into consume(t):
        # needs its own double buffer
        bc = smallpool.tile([P, QT], f32, tag="bc", name="bc", bufs=2)
        nc.vector.reciprocal_approx_fast(bc[:], pr["sums"][:])
        pr["bc"] = bc

    def sums_fin(pr):
        sums_mm(pr)
        sums_recip(pr)

    def consume_out(pr, then_pr=None, qoff=0, qw=QT):
        """B-phase: context/output matmuls -> normalize -> bias-add + square
        drains -> m2, for columns [qoff, qoff+qw). then_pr: the next tile's
        sums finalization, injected after the context matmuls (PE) / after
        m2add (DVE). Tiles allocate on the first call and are shared by
        later column-slices (tail split)."""
        st, qt, bh = pr["st"], pr["qt"], pr["bh"]
        attn, bc = pr["attn"], pr["bc"]
        v_r, v_i = st["v_r"], st["v_i"]
        csl = slice(qoff, qoff + qw)
        with nc.named_scope(f"out{bh}_{qt}"):
            if "out_r" not in pr:
                pr["out_r"] = outppool.tile([P, DC, QT], f32, tag="out_r",
                                            name="out_r")
                pr["out_i"] = outppool.tile([P, DC, QT], f32, tag="out_i",
                                            name="out_i")
                pr["m2"] = m2pool.tile([P, DC, QT], f32, tag="m2r",
                                       name="m2r", bufs=2)
                pr["m2i"] = m2pool.tile([P, DC, QT], f32, tag="m2i",
                                        name="m2i")
                pr["tmp"] = smallpool.tile([P, DC, QT], f32, tag="tmp",
                                           name="tmp")
            out_r = pr["out_r"][:, :, csl]
            out_i = pr["out_i"][:, :, csl]
            m2r = pr["m2"][:, :, csl]
            m2i = pr["m2i"][:, :, csl]
            tmp = pr["tmp"][:, :, csl]
            for dst, m2, vsrc, b in ((out_r, m2r, v_r, "bor"),
                                     (out_i, m2i, v_i, "boi")):
                for c in range(DC):
                    cps = ps_out.tile([P, QT], f32, tag="outp",
                                      name="outp")[:, :qw]
                    for kc in range(NKC):
                        nc.tensor.matmul(cps, vsrc[:, kc, c * P:(c + 1) * P],
                                         attn[:, kc, csl],
                                         start=(kc == 0), stop=(kc == NKC - 1),
                                         skip_group_check=True)
                    # normalize, then bias-add on ACT (Identity) + square (for
                    # the magnitude) in the same table
                    nc.vector.tensor_mul(tmp[:, c, :], cps, bc[:, csl])
                    nc.scalar.activation(dst[:, c, :], tmp[:, c, :],
                                         AF.Identity, bias=bias[b][:, c:c + 1])
                    nc.scalar.activation(m2[:, c, :], tmp[:, c, :],
                                         AF.Square, bias=bias[b][:, c:c + 1])

            if then_pr is not None:
                sums_mm(then_pr)  # PE: right after the context matmuls

            nc.vector.tensor_add(m2r[:], m2r[:], m2i[:])
            if then_pr is not None:
                sums_recip(then_pr)  # DVE: after m2add

    def consume_gate(pr, qoff=0, qw=QT):
        """C-phase (deferred one pipeline stage so the ACT table switches for
        Sqrt sit under the next tile's PE work): mag -> gate matmul ->
        sigmoid -> gated store, for columns [qoff, qoff+qw)."""
        qt, bh = pr["qt"], pr["bh"]
        out_r = pr["out_r"][:, :, qoff:qoff + qw]
        out_i = pr["out_i"][:, :, qoff:qoff + qw]
        m2 = pr["m2"][:, :, qoff:qoff + qw]
        with nc.named_scope(f"gate{bh}_{qt}"):
            mag = smallpool.tile([P, DC, QT], fp16, tag="mag", name="mag")[:, :, :qw]
            nc.scalar.activation(mag[:], m2[:], AF.Sqrt, bias=eps_t[:])

            gate = smallpool.tile([P, DC, QT], f32, tag="gate", name="gate")[:, :, :qw]
            eg = smallpool.tile([P, DC, QT], f32, tag="eg", name="eg")[:, :, :qw]
            for go in range(DC):
                gps = ps_mm.tile([P, QT], f32, tag="mm", name="mm")[:, :qw]
                nc.tensor.matmul(gps, w["gwT"][:, 0, go * P:(go + 1) * P],
                                 mag[:, 0, :], start=True, stop=False)
                nc.tensor.matmul(gps, w["gwT"][:, 1, go * P:(go + 1) * P],
                                 mag[:, 1, :], start=False, stop=True)
                # sigmoid(z+bg) = 1/(1 + exp(-z-bg)): exp on ACT (exp table),
                # +1 and reciprocal on DVE
                nc.scalar.activation(eg[:, go, :], gps, AF.Exp, scale=-1.0,
                                     bias=bias["bgn"][:, go:go + 1])
            nc.vector.tensor_scalar_add(eg[:], eg[:], 1.0)
            nc.vector.reciprocal_approx_fast(gate[:], eg[:])

            nc.vector.tensor_mul(out_r[:], out_r[:], gate[:])
            nc.vector.tensor_mul(out_i[:], out_i[:], gate[:])
            osl = slice(qt * QT + qoff, qt * QT + qoff + qw)
            for c in range(DC):
                nc.sync.dma_start(tens["yr"][bh, c, :, osl], out_r[:, c, :])
                nc.sync.dma_start(tens["yi"][bh, c, :, osl], out_i[:, c, :])

    # ---- software-pipelined emission ---------------------------------------
    # steady state per cycle: produce(t) | B(t-1) [+ sums(t)] | C(t-2)
    pending = None       # produced, awaiting B
    pending_gate = None  # B done, awaiting C
    lds = {0: load_inputs(0, first=True)}
    for bh in range(BH_PER_CORE):
        ld = lds[bh]
        u_r, u_i = proj_u(bh, ld)
        v_r, v_i = proj_veff(bh, ld)
        st = {"bh": bh, "u_r": u_r, "u_i": u_i, "v_r": v_r, "v_i": v_i,
              "xkr": ld["xkr"], "xki": ld["xki"]}
        for qt in range(NQT):
            if bh == 0 and qt == 1:
                continue  # produced during warm-up below
            pr = produce(st, qt)
            if pending is None:
                # warm-up: emit the next produce before the first sums
                # finalization so the reduce-chain latency hides behind PE work
                pr2 = produce(st, 1)
                sums_fin(pr)
                consume_out(pr, then_pr=pr2)
                pending, pending_gate = pr2, pr
                continue
            if qt == 1 and bh + 1 < BH_PER_CORE:
                lds[bh + 1] = load_inputs(bh + 1)  # prefetch next (b,h)
            consume_out(pending, then_pr=pr)
            consume_gate(pending_gate)
            pending_gate, pending = pending, pr
        if bh == 0 and BH_PER_CORE > 1:
            lds[1] = load_inputs(1)  # prefetch (emitted after (0,3) produce)
    # drain the pipeline: split the final out-phase into column halves and
    # interleave the gate phases so PE work covers the ACT table switches
    consume_out(pending, qoff=0, qw=QT // 2)
    consume_gate(pending_gate)
    consume_out(pending, qoff=QT // 2, qw=QT // 2)
    consume_gate(pending, qoff=0, qw=QT // 2)
    consume_gate(pending, qoff=QT // 2, qw=QT // 2)


def _build():
    global _BUILT
    if _BUILT is not None:
        return _BUILT
    from contextlib import ExitStack
    import concourse.tile as tile
    from concourse import bacc, mybir

    f32 = mybir.dt.float32
    fp16 = mybir.dt.float16

    nc = bacc.Bacc("TRN2", target_bir_lowering=False, debug=False,
                   num_devices=NCORES)

    tens = {}
    for name in ("xq_r", "xq_i", "xkv_r", "xkv_i"):
        tens[name] = nc.dram_tensor(name, [BH_PER_CORE, P, DC, T], fp16,
                                    kind="ExternalInput").ap()
    tens["wpack"] = nc.dram_tensor("wpack", [P, DC, NW * D], fp16,
                                   kind="ExternalInput").ap()
    tens["bpack"] = nc.dram_tensor("bpack", [P, NB * DC], f32,
                                   kind="ExternalInput").ap()
    for name in ("yr", "yi"):
        tens[name] = nc.dram_tensor(name, [BH_PER_CORE, DC, P, T], f32,
                                    kind="ExternalOutput").ap()

    with tile.TileContext(nc) as tc:
        with ExitStack() as ctx:
            _emit_kernel(nc, tc, ctx, tens)

    nc.compile()
    _BUILT = nc
    return nc


def _lhsT_pack(m):
    """[din, dout] stationary weight -> [P, DC, D] fp16 (din chunked,
    partition-major so the DMA is contiguous)."""
    return np.ascontiguousarray(
        m.reshape(DC, P, D).transpose(1, 0, 2).astype(F16))


def _bias_pack(b):
    """[D] bias -> [P, DC] f32 (per-partition d-major layout)."""
    return np.ascontiguousarray(np.asarray(b).reshape(DC, P).T.astype(np.float32))


def _x_pack(x):
    """[BH, T, D] fp32 -> [BH, P, DC, T] fp16 (d-major, partition-major so
    the DMA is contiguous per partition)."""
    xb = x.astype(F16)
    return np.ascontiguousarray(xb.reshape(BH, T, DC, P).transpose(0, 3, 2, 1))


def kernel(**inputs):
    inputs = {k: np.asarray(v) for k, v in inputs.items()}

    nc = _build()
    from concourse.bass_utils import run_bass_kernel_spmd

    xq_r = _x_pack(inputs["q_in_r"].reshape(BH, T, D))
    xq_i = _x_pack(inputs["q_in_i"].reshape(BH, T, D))
    xkv_r = _x_pack(inputs["kv_in_r"].reshape(BH, T, D))
    xkv_i = _x_pack(inputs["kv_in_i"].reshape(BH, T, D))

    f64 = np.float64
    qwr = inputs["q_wr"].astype(f64); qwi = inputs["q_wi"].astype(f64)
    kwr = inputs["k_wr"].astype(f64); kwi = inputs["k_wi"].astype(f64)
    vwr = inputs["v_wr"].astype(f64); vwi = inputs["v_wi"].astype(f64)
    owr = inputs["o_wr"].astype(f64); owi = inputs["o_wi"].astype(f64)
    gw = inputs["gate_w"].astype(f64)

    # u-projection: scores^T = (xq@A + xq_i@Bm ...) against raw kv inputs
    A = qwr.T @ kwr + qwi.T @ kwi
    Bm = qwr.T @ kwi - qwi.T @ kwr
    # veff: W_eff = W_o W_v (complex product)
    Wer = owr @ vwr - owi @ vwi
    Wei = owr @ vwi + owi @ vwr

    wmats = {"A": A, "Bm": Bm, "Bmn": -Bm,
             "WerT": Wer.T, "WeiT": Wei.T, "WeinT": -Wei.T, "gwT": gw.T}
    wpack = np.concatenate([_lhsT_pack(wmats[n]) for n in WNAMES], axis=-1)

    # u bias rows (b_q folded through the k weights; b_k drops: it only adds
    # per-q constants to scores, which softmax ignores)
    bqr = inputs["q_br"].astype(f64); bqi = inputs["q_bi"].astype(f64)
    bur = bqr @ kwr + bqi @ kwi
    bui = bqi @ kwr - bqr @ kwi
    # out bias: b_out = W_o b_v + b_o (v bias survives softmax row-sums = 1)
    vbr = inputs["v_br"].astype(f64); vbi = inputs["v_bi"].astype(f64)
    bor = inputs["o_br"].astype(f64) + owr @ vbr - owi @ vbi
    boi = inputs["o_bi"].astype(f64) + owi @ vbr + owr @ vbi

    bmats = {"bur": bur, "bui": bui, "bor": bor, "boi": boi,
             "bgn": -inputs["gate_b"].astype(f64)}
    bpack = np.concatenate([_bias_pack(bmats[n]) for n in BNAMES], axis=-1)

    consts = {"wpack": np.ascontiguousarray(wpack),
              "bpack": np.ascontiguousarray(bpack)}

    in_maps = []
    for c in range(NCORES):
        sl = slice(c * BH_PER_CORE, (c + 1) * BH_PER_CORE)
        m = dict(consts)
        m["xq_r"] = xq_r[sl]
        m["xq_i"] = xq_i[sl]
        m["xkv_r"] = xkv_r[sl]
        m["xkv_i"] = xkv_i[sl]
        in_maps.append(m)

    res = run_bass_kernel_spmd(nc, in_maps, core_ids=list(range(NCORES)))

    def unpack(name):
        full = np.concatenate([res.results[c][name] for c in range(NCORES)], axis=0)
        # [BH, DC, P, T] -> [BH, T, DC*P] -> [B, H, T, D]
        return np.ascontiguousarray(
            full.transpose(0, 3, 1, 2).reshape(B, H, T, D).astype(np.float32))

    return unpack("yr"), unpack("yi")


if __name__ == "__main__":
    # smoke test with random inputs
    rng = np.random.default_rng(0)
    fake = {}
    for nm in ("q_in_r", "q_in_i", "kv_in_r", "kv_in_i"):
        fake[nm] = rng.standard_normal((B, H, T, D), dtype=np.float32)
    for p in ("q", "k", "v", "o"):
        fake[f"{p}_wr"] = rng.standard_normal((D, D), dtype=np.float32) * 0.044
        fake[f"{p}_wi"] = rng.standard_normal((D, D), dtype=np.float32) * 0.044
        fake[f"{p}_br"] = np.zeros(D, np.float32)
        fake[f"{p}_bi"] = np.zeros(D, np.float32)
    fake["gate_w"] = rng.standard_normal((D, D), dtype=np.float32) * 0.044
    fake["gate_b"] = np.zeros(D, np.float32)
    yr, yi = kernel(**fake)
    print("OK", yr.shape, yi.shape, yr.dtype)
